# revision 1
# baseline (speedup 1.0000x reference)
"""DiagonalLSTM Trainium2 kernel.

Sharding: data-parallel over batch B=16 across 8 cores (2 batch elems/core).
Per-core layout: partitions = 128-wide HID gate chunks, free dim = (b, h).

Per scan step t (127 steps), each of the 5 gate chunks accumulates in PSUM:
    wis_aug @ x_diag   (K=65: 64 channels + ones row folding b_is+b_ss)
  + w0_chunk @ h_prev  written column-shifted by one H position
  + w1_chunk @ h_prev
All matmuls fp32: the scan dynamics chaotically amplify per-step rounding
noise, so bf16/fp16 inputs fail; fp32 matches the fp32-reference envelope.

Sigmoid gates computed as 0.5*(1+tanh(x/2)) — ACT tanh is ~2 ULP vs ~20 ULP
native sigmoid, which matters under the chaotic amplification. The sigmoid
chunks' weights/biases are pre-halved on the host so ONE tanh activation over
all 5 gate chunks serves both the 4 sigmoid gates and the g-gate.

The x-side matmuls for step t+1 are emitted right after step t's tap matmuls
so the PE stays busy while ACT/DVE run step t's nonlinear/elementwise chain.
h is accumulated into a residual tile along stride-63 diagonal APs; one DMA
out at the end.
"""

import numpy as np

import concourse.bass as bass
import concourse.mybir as mybir
from concourse import bacc
from concourse import tile
from concourse.bass_utils import run_bass_kernel_spmd

B, C, H, W = 16, 64, 64, 64
HID = 128
SW = H + W - 1  # 127
NCORES = 8
BL = B // NCORES  # 2
NBH = BL * H      # 128 free columns (b, h)
NXC = BL * H * SW  # 16256 skewed cols
NRES = BL * H * W  # 8192 output cols

F32 = mybir.dt.float32
AF = mybir.ActivationFunctionType
ALU = mybir.AluOpType

# Use single matmuls with 2D free APs spanning both batch blocks (fewer
# instructions and, more importantly, fewer LDWEIGHTS on hardware).  The
# executing simulator can't evaluate 2D-free matmuls (shape assert), so
# dbg_sim sets this False and rebuilds for numerics validation — the two
# forms are mechanically equivalent.
B2D = True


def _raw(t, off, dims):
    """Raw AP on tile t: keep its partition pair, custom free dims."""
    return bass.AP(t.tensor, t.offset + off, [list(t.ap[0])] + [list(d) for d in dims])


def build_program():
    nc = bacc.Bacc(None, target_bir_lowering=False)

    xsk_d = nc.dram_tensor("xsk", [C, NXC], F32, kind="ExternalInput")
    xres_d = nc.dram_tensor("xres", [C + 1, NRES], F32, kind="ExternalInput")
    wtap_d = nc.dram_tensor("wtap", [HID, 2 * 5 * HID], F32, kind="ExternalInput")
    wis_d = nc.dram_tensor("wis", [C, 5 * HID], F32, kind="ExternalInput")
    wres_d = nc.dram_tensor("wres", [C + 1, HID], F32, kind="ExternalInput")
    bias_d = nc.dram_tensor("bias", [HID, 5], F32, kind="ExternalInput")
    out_d = nc.dram_tensor("out", [HID, NRES], F32, kind="ExternalOutput")

    with tile.TileContext(nc) as tc:
        with (
            tc.tile_pool(name="const", bufs=1) as const,
            tc.tile_pool(name="state", bufs=3) as state,
            tc.tile_pool(name="tmp", bufs=3) as tmp,
            tc.tile_pool(name="gpsum", bufs=8, space="PSUM") as gpsum,
        ):
            xsk = const.tile([C, NXC], F32)
            xres = const.tile([C + 1, NRES], F32)
            wtap = const.tile([HID, 2 * 5 * HID], F32)
            wis = const.tile([C, 5 * HID], F32)
            wres = const.tile([C + 1, HID], F32)
            bias = const.tile([HID, 5], F32)
            res = const.tile([HID, NRES], F32)

            # weights on the sync queue; xsk t-major ([c, t, b, r]) in
            # chunks on the gpsimd queue (parallel with the weights) so the
            # scan starts as soon as the first steps' columns land; xres
            # (needed from step 70) last.
            nc.sync.dma_start(out=wis, in_=wis_d[:])
            nc.sync.dma_start(out=bias, in_=bias_d[:])
            nc.sync.dma_start(out=wtap, in_=wtap_d[:])
            nc.sync.dma_start(out=wres, in_=wres_d[:])
            cuts = [0, 4 * NBH, 16 * NBH] + list(
                range(32 * NBH, NXC, 16 * NBH)
            ) + [NXC]
            for lo, hi in zip(cuts[:-1], cuts[1:]):
                nc.sync.dma_start(out=xsk[:, lo:hi], in_=xsk_d[:, lo:hi])
            nc.sync.dma_start(out=xres, in_=xres_d[:])

            # Each PSUM accumulation group gets its own one-bank pool tile;
            # the 8-deep pool recycles banks round-robin, so at most one
            # open group per bank (HW zero-region constraint) and next-step
            # x-matmuls open groups in banks whose previous group was
            # consumed ~1.6 steps earlier.
            def pbank():
                ps = gpsum.tile([HID, 512], F32, tag="ps")
                return ps

            # ---- scan state ----
            h_cur = state.tile([HID, NBH], F32, tag="h")
            c_cur = state.tile([HID, NBH], F32, tag="c")
            nc.vector.memzero(h_cur)
            nc.vector.memzero(c_cur)

            # scan chunk emission order (gate chunk index k): fl, fu, i, g, o
            KORD = (1, 2, 3, 4, 0)

            def xmm(t):
                """i_s matmuls for step t (opens the 5 groups).

                Only the x-valid window [max(0,t-63) .. min(t,63)] is
                computed: below it the skew is zero (i_s contribution = 0,
                bias arrives via the ACT bias port), above it rows are dead
                (t >= W).  PSUM cols not written here are zeroed by the tap
                matmuls (pending-zero semantics).
                """
                vlo = max(0, t - (W - 1))
                base = t * NBH
                tiles = []
                for k in KORD:
                    pk = pbank()[:, 0:NBH]
                    wc = wis[:, k * HID:(k + 1) * HID]
                    if vlo == 0:
                        nc.tensor.matmul(
                            pk, wc, xsk[:, base:base + NBH],
                            start=True, stop=False,
                        )
                    elif B2D:
                        pkv = pk.rearrange("p (b r) -> p b r", b=BL)[:, :, vlo:]
                        xv = xsk[:, base:base + NBH].rearrange(
                            "p (b r) -> p b r", b=BL
                        )[:, :, vlo:]
                        nc.tensor.matmul(pkv, wc, xv, start=True, stop=False)
                    else:
                        for b in range(BL):
                            nc.tensor.matmul(
                                pk[:, b * H + vlo:(b + 1) * H],
                                wc, xsk[:, base + b * H + vlo:base + (b + 1) * H],
                                start=(b == 0), stop=False,
                            )
                    tiles.append(pk)
                return tiles

            pcur = xmm(0)

            # State convention: h_cur holds 2h, c_cur holds 2c.  Sigmoid
            # gates are T/2 with T = tanh(z/2)+1 (weights pre-halved on the
            # host; tap weights additionally halved to absorb the 2h).
            #   C2'   = 0.5*(T_fl*C2 + T_fu*C2sh) + T_i*g
            #   H2'   = T_o * tanh(C2'/2)
            #   res  += 0.5*H2'
            # Tap matmuls are emitted in chunk order fl,fu,i,g,o so the gate
            # tanh can run in three slices overlapping the remaining taps.
            for t in range(SW):
                lo = max(0, t - (W - 1))

                def V(ap, a=None):
                    """Live-range view [lo..H) of each batch block (3D)."""
                    s = lo if a is None else a
                    return ap.rearrange("p (b r) -> p b r", b=BL)[:, :, s:H]

                th = tmp.tile([HID, 5 * HID], F32, tag="th")
                for idx, k in enumerate(KORD):
                    pk = pcur[idx]
                    w0c = wtap[:, k * HID:(k + 1) * HID]
                    w1c = wtap[:, 5 * HID + k * HID: 5 * HID + (k + 1) * HID]
                    # w0 @ h_prev, H-shifted, live rows only
                    s0 = max(lo, 1)
                    if B2D:
                        nc.tensor.matmul(
                            V(pk, s0), w0c,
                            h_cur.rearrange("p (b r) -> p b r", b=BL)[:, :, s0 - 1:H - 1],
                            start=False, stop=False,
                        )
                    else:
                        for b in range(BL):
                            nc.tensor.matmul(
                                pk[:, b * H + s0:(b + 1) * H],
                                w0c, h_cur[:, b * H + s0 - 1:(b + 1) * H - 1],
                                start=False, stop=False,
                            )
                    # w1 @ h_prev
                    if lo == 0:
                        nc.tensor.matmul(pk, w1c, h_cur, start=False, stop=True)
                    elif B2D:
                        nc.tensor.matmul(
                            V(pk), w1c, V(h_cur), start=False, stop=True
                        )
                    else:
                        for b in range(BL):
                            nc.tensor.matmul(
                                pk[:, b * H + lo:(b + 1) * H],
                                w1c, h_cur[:, b * H + lo:(b + 1) * H],
                                start=False, stop=(b == BL - 1),
                            )
                    # per-chunk tanh (+ per-partition gate bias): fires as
                    # soon as this chunk's bank is complete, overlapping
                    # the remaining taps
                    thc = th[:, idx * HID:(idx + 1) * HID]
                    bk = bias[:, k:k + 1]
                    if lo == 0:
                        nc.scalar.activation(thc, pk, AF.Tanh, bias=bk)
                    else:
                        nc.scalar.activation(V(thc), V(pk), AF.Tanh, bias=bk)

                # prefetch next step's x-side matmuls while ACT/DVE run
                if t + 1 < SW:
                    pcur = xmm(t + 1)

                t_fl = th[:, 0:HID]
                t_fu = th[:, HID:2 * HID]
                t_i = th[:, 2 * HID:3 * HID]
                g = th[:, 3 * HID:4 * HID]
                t_o = th[:, 4 * HID:5 * HID]

                # P = (t_fl+1)*C2 ; P += (t_fu+1)*C2sh (rows >= max(lo,1));
                # C2' = 0.5*P + (t_i+1)*g   — all on live rows [lo..H)
                p = tmp.tile([HID, NBH], F32, tag="p")
                nc.vector.scalar_tensor_tensor(
                    V(p), V(t_fl), 1.0, V(c_cur), op0=ALU.add, op1=ALU.mult
                )
                q = tmp.tile([HID, NBH], F32, tag="q")
                s0 = max(lo, 1)
                cc_sh = c_cur.rearrange("p (b r) -> p b r", b=BL)[:, :, s0 - 1:H - 1]
                nc.vector.scalar_tensor_tensor(
                    V(q, s0), V(t_fu, s0), 1.0, cc_sh,
                    op0=ALU.add, op1=ALU.mult,
                )
                nc.vector.tensor_add(V(p, s0), V(p, s0), V(q, s0))
                r_t = tmp.tile([HID, NBH], F32, tag="r_t")
                nc.vector.scalar_tensor_tensor(
                    V(r_t), V(t_i), 1.0, V(g), op0=ALU.add, op1=ALU.mult
                )
                c_new = state.tile([HID, NBH], F32, tag="c")
                nc.vector.scalar_tensor_tensor(
                    V(c_new), V(p), 0.5, V(r_t), op0=ALU.mult, op1=ALU.add
                )

                tanc = tmp.tile([HID, NBH], F32, tag="tanc")
                nc.scalar.activation(V(tanc), V(c_new), AF.Tanh, scale=0.5)
                h_new = state.tile([HID, NBH], F32, tag="h")
                nc.vector.scalar_tensor_tensor(
                    V(h_new), V(t_o), 1.0, V(tanc), op0=ALU.add, op1=ALU.mult
                )

                # write H2 into res along the diagonal w = t - r (gpsimd:
                # keeps DVE free).  Each res cell is touched exactly once,
                # so this is a copy — no init needed.  res holds
                # 2*(residual + h); the host halves the output.
                rlo = max(0, t - (W - 1))
                rhi = min(H - 1, t)
                nr = rhi - rlo + 1
                res_ap = _raw(
                    res, (W - 1) * rlo + t, [[H * W, BL], [W - 1, nr]]
                )
                h_ap = h_new.rearrange("p (b r) -> p b r", b=BL)[:, :, rlo:rhi + 1]
                nc.gpsimd.tensor_copy(out=res_ap, in_=h_ap)

                h_cur = h_new
                c_cur = c_new

                # Late-scan interleave: once an 8-row block's diagonal cells
                # are all written (t = 8j+70), add its residual
                # (w_res @ x + b_res, doubled on host) and DMA it out.
                # These fill the PE/DVE idle left by the shrinking tail.
                if t >= 70 and (t - 70) % 8 == 0 and (t - 70) // 8 < 8:
                    j = (t - 70) // 8
                    for b in range(BL):
                        cols = slice(b * H * W + 512 * j, b * H * W + 512 * j + 512)
                        rp = pbank()
                        nc.tensor.matmul(
                            rp, wres, xres[:, cols], start=True, stop=True
                        )
                        nc.vector.tensor_add(res[:, cols], res[:, cols], rp)
                        nc.sync.dma_start(out=out_d[:, cols], in_=res[:, cols])

    nc.finalize()
    return nc


_NC_CACHE = {}


def _get_nc():
    if "nc" not in _NC_CACHE:
        _NC_CACHE["nc"] = build_program()
    return _NC_CACHE["nc"]


def _prep_inputs(x, w_is, b_is, w_ss, b_ss, w_res, b_res):
    x = np.asarray(x, np.float32)
    # skewed x: [B, C, H, SW], row r shifted right by r
    sk = np.zeros((B, C, H, SW), np.float32)
    for r in range(H):
        sk[:, :, r, r:r + W] = x[:, :, r, :]
    # [core, c, t, b, r] (t-major so each step reads a contiguous slice)
    xsk = sk.reshape(NCORES, BL, C, H, SW).transpose(0, 2, 4, 1, 3)
    xsk = np.ascontiguousarray(xsk).reshape(NCORES, C, NXC)

    xres = np.asarray(x).reshape(NCORES, BL, C, H, W).transpose(0, 2, 1, 3, 4)
    xres = xres.reshape(NCORES, C, NRES)
    xres = np.concatenate([xres, np.ones((NCORES, 1, NRES), np.float32)], axis=1)

    # gate scaling: chunks 0..3 (o, f_left, f_up, i) are sigmoid gates,
    # computed via tanh(z/2) -> pre-halve their weights and biases.
    gs = np.ones((5 * HID,), np.float32)
    gs[0:4 * HID] = 0.5

    # wtap[i, tap*640 + o] = w_ss[o, i, tap] * gs[o] * 0.5
    # (extra 0.5: the kernel's h state holds 2h)
    wtap = np.asarray(w_ss, np.float32).transpose(1, 2, 0) * (0.5 * gs)[None, None, :]
    wtap = np.ascontiguousarray(wtap.reshape(HID, 2 * 5 * HID), np.float32)
    wis = np.ascontiguousarray(
        np.asarray(w_is, np.float32).T * gs[None, :], np.float32
    )
    # per-partition gate bias, fed through the ACT bias port: [128, 5]
    bvec = (np.asarray(b_is, np.float32) + np.asarray(b_ss, np.float32)) * gs
    biases = np.ascontiguousarray(bvec.reshape(5, HID).T, np.float32)
    # x2: the device residual tile accumulates 2*(residual + sum h); the
    # host halves the final output.
    wres = 2.0 * np.concatenate(
        [np.asarray(w_res, np.float32).T, np.asarray(b_res, np.float32)[None, :]],
        axis=0,
    ).astype(np.float32)

    in_maps = []
    for c in range(NCORES):
        in_maps.append({
            "xsk": np.ascontiguousarray(xsk[c]),
            "xres": np.ascontiguousarray(xres[c]),
            "wtap": wtap,
            "wis": wis,
            "wres": wres,
            "bias": biases,
        })
    return in_maps


def kernel(x, w_is, b_is, w_ss, b_ss, w_res, b_res, _trace=False):
    nc = _get_nc()
    in_maps = _prep_inputs(x, w_is, b_is, w_ss, b_ss, w_res, b_res)
    r = run_bass_kernel_spmd(nc, in_maps, list(range(NCORES)), trace=_trace)
    outs = [r.results[c]["out"] for c in range(NCORES)]
    out = np.stack(outs, 0).reshape(NCORES, HID, BL, H, W)
    out = out.transpose(0, 2, 1, 3, 4).reshape(B, HID, H, W)
    return np.ascontiguousarray(out * np.float32(0.5))

